# revision 1
# baseline (speedup 1.0000x reference)
"""Multi-head attention (B=2, S=2048, D=1024, H=16, dk=dv=64) on 8 TRN2 cores.

Sharding: core c -> batch b = c % 2, head-group g = c // 2 (heads 4g..4g+3).
Each core computes its 4 heads' attention for one batch plus the partial
output projection; the host sums the 4 partials per batch and adds bo.

Host marshalling: inputs are sliced per batch, transposed to [D, S]
(the PE contracts over the partition dim, so projections need D-major
operands), and the per-head weights are packed/stacked; the reference's
softmax/dk/2 scale is folded into Wv and bv.

Per-core device pipeline (matmuls in float32r: full rate, ~13-bit mantissa):
  1. QWT/KWT [dk, S] head projections (heads pair-stacked on partitions,
     biases fused into the ACT PSUM->SBUF eviction), VW [S, dv] natural
     (bias + the softmax-denominator ones column via K=1 rank-1 matmuls).
  2. scoresT[t, s] = KWT.T @ QWT per head, two heads concurrently via
     64x128 PE row tiling; exp fused into the PSUM->SBUF eviction (ACT).
     No max-subtraction (|scores| < 40, exp stays finite in fp32).
  3. ctxT[dv+1, s] = VW1.T @ exp_scoresT accumulated over t; row dv is the
     softmax denominator. Normalize: K=1 matmul broadcasts the denominator
     row to 64 partitions, DVE reciprocal, DVE multiply (the eviction).
  4. out[s, D] partial = ctx_allT.T @ Wo_slice, ACT-evicted, DMA'd out.
"""
import os
import sys

sys.path.insert(0, "/opt/trn_rl_repo")
os.environ.setdefault("JAX_PLATFORMS", "axon,cpu")

from contextlib import ExitStack

import numpy as np

import concourse.bacc as bacc
import concourse.tile as tile
from concourse import mybir
from concourse.bass_utils import run_bass_kernel_spmd

FP32 = mybir.dt.float32
FP32R = mybir.dt.float32r

B, S, D = 2, 2048, 1024
H, DK, DV = 16, 64, 64
N_CORES = 8
HPC = H // (N_CORES // B)  # heads per core = 4
P = 128
SBLK = 512                # s-block (free dim of scores matmuls)
NBLK = S // SBLK          # 4
NTT = S // P              # 16 t-tiles
NDC = D // P              # 8 contraction chunks
NV = HPC * (DV + 1)       # 260
SCALE = 1.0 / (DK * 2.0)  # folded into Wv/bv


def _build_nc():
    nc = bacc.Bacc("TRN2", target_bir_lowering=False, debug=False,
                   num_devices=N_CORES)
    d = {}
    for name, shape in [
        ("qt", [D, S]), ("kt", [D, S]), ("vt", [D, S]),
        ("wq", [D, 2 * P]), ("wk", [D, 2 * P]), ("wv", [D, 2 * P]),
        ("bqk", [P, 6]), ("ident", [P, P]), ("onescol", [P, NTT]),
        ("wo", [HPC * DV, D]), ("ones", [1, SBLK]),
    ]:
        d[name] = nc.dram_tensor(name, shape, FP32, kind="ExternalInput").ap()
    out_d = nc.dram_tensor("out", [S, D], FP32, kind="ExternalOutput").ap()
    # [D, S] viewed as [p, dc, s] chunks for DMA
    xt_view = {
        n: d[n].rearrange("(dc p) s -> p dc s", p=P).bitcast(FP32R)
        for n in ("qt", "kt", "vt")
    }

    with tile.TileContext(nc) as tc, ExitStack() as ctx:
        const = ctx.enter_context(tc.tile_pool(name="const", bufs=1))
        wpool = ctx.enter_context(tc.tile_pool(name="wpool", bufs=1))
        xtp = ctx.enter_context(tc.tile_pool(name="xtp", bufs=2))
        projp = ctx.enter_context(tc.tile_pool(name="projp", bufs=1))
        expp = ctx.enter_context(tc.tile_pool(name="expp", bufs=1))
        ctxp = ctx.enter_context(tc.tile_pool(name="ctxp", bufs=1))
        outp = ctx.enter_context(tc.tile_pool(name="outp", bufs=2))
        smallp = ctx.enter_context(tc.tile_pool(name="smallp", bufs=2))
        psum = ctx.enter_context(tc.tile_pool(name="psum", bufs=1, space="PSUM"))

        # ---- constants / weights (wk first: K projection starts the kernel;
        # the rest trickle in behind the first K chunk loads) ----
        wk_sb = wpool.tile([P, NDC, 2 * P], FP32R)
        nc.sync.dma_start(wk_sb[:], d["wk"].rearrange("(dc p) m -> p dc m", p=P).bitcast(FP32R))
        bqk = const.tile([P, 6], FP32)
        nc.sync.dma_start(bqk[:], d["bqk"])
        ones_r = const.tile([1, SBLK], FP32R)
        nc.sync.dma_start(ones_r[:], d["ones"].bitcast(FP32R))
        ident_r = const.tile([P, P], FP32R)
        nc.sync.dma_start(ident_r[:], d["ident"].bitcast(FP32R))
        wv_sb = wpool.tile([P, NDC, 2 * P], FP32R)
        wq_sb = wpool.tile([P, NDC, 2 * P], FP32R)
        wo_sb = wpool.tile([P, 2, D], FP32R)

        def load_w(sb, name, pat="(dc p) m -> p dc m"):
            nc.sync.dma_start(sb[:], d[name].rearrange(pat, p=P).bitcast(FP32R))

        # ---- persistent activation tiles ----
        qwt = [projp.tile([P, S], FP32R, tag=f"qwt{p_}", name=f"qwt{p_}") for p_ in range(2)]
        kwt = [projp.tile([P, S], FP32R, tag=f"kwt{p_}", name=f"kwt{p_}") for p_ in range(2)]
        vwt = [projp.tile([P, S], FP32R, tag=f"vwt{p_}", name=f"vwt{p_}") for p_ in range(2)]
        vw = projp.tile([P, NTT, NV], FP32R, tag="vw")
        # softmax-denominator ones column (once, strided over the 65-wide head slots)
        for hh in range(HPC):
            nc.sync.dma_start(vw[:, :, hh * (DV + 1) + DV],
                              d["onescol"].bitcast(FP32R))
        ctx_t = [ctxp.tile([P, S], FP32R, tag=f"ctx{p_}", name=f"ctx{p_}") for p_ in range(2)]

        def load_chunk(name, ci, tag="xtk", bufs=2):
            xt = xtp.tile([P, NDC, SBLK], FP32R, tag=tag, name="xt", bufs=bufs)
            nc.sync.dma_start(xt[:], xt_view[name][:, :, ci * SBLK:(ci + 1) * SBLK])
            return xt

        def proj_qk_pair(xt, w_sb, dst, bias_col, ci, pair, tag=None):
            """Project one head-pair of a chunk into dst[pair][:, ci*SBLK:...]."""
            pq = psum.tile([P, SBLK], FP32, tag=tag or ("pj" if pair == 0 else "po"),
                           name="pq")
            for dc in range(NDC):
                nc.tensor.matmul(pq[:], lhsT=w_sb[:, dc, pair * P:(pair + 1) * P],
                                 rhs=xt[:, dc, :], start=(dc == 0), stop=(dc == NDC - 1))
            nc.scalar.activation(dst[pair][:, ci * SBLK:(ci + 1) * SBLK], pq[:],
                                 mybir.ActivationFunctionType.Identity,
                                 bias=bqk[:, bias_col + pair:bias_col + pair + 1])

        def proj_v(xt, ci):
            """VWT (pair-stacked, like Q/K), then PE-transpose into vw natural."""
            for pair in range(2):
                proj_qk_pair(xt, wv_sb, vwt, 4, ci, pair)
            for pair in range(2):
                for c in range(SBLK // P):
                    tt = ci * (SBLK // P) + c
                    tp = psum.tile([P, P], FP32R, tag="ct0" if (pair * 4 + c) % 2 == 0 else "ct1",
                                   name="tp")
                    nc.tensor.transpose(
                        tp[:], vwt[pair][:, ci * SBLK + c * P:ci * SBLK + (c + 1) * P],
                        ident_r[:])
                    nc.vector.tensor_copy(
                        vw[:, tt, :].rearrange("p (h v) -> p h v", v=DV + 1)[:, 2 * pair:2 * pair + 2, 0:DV],
                        tp[:].rearrange("p (h v) -> p h v", h=2))

        def attn_alloc(pair):
            return [psum.tile([DV + 1, SBLK], FP32, tag=f"ct{hp}", name=f"ct{hp}")
                    for hp in range(2)]

        def attn_block(pair, b, ct, fillers):
            """Per-2-t-tile pipeline: scores(k) -> exp(k) -> ctx(k), ctx chasing
            exp by one step. One 4-bank scores PSUM per step holds both heads'
            2 t-tiles, evicted by a single FD=2048 exp. `fillers` is a list of
            no-arg callables emitting extra PE work, drained one per step."""
            NK = NTT // 2
            exs = {}
            for k in range(NK + 2):
                if k < NK:
                    sc = [psum.tile([P, 2 * SBLK], FP32, tag=f"sc{hp}", name=f"sc{hp}")
                          for hp in range(2)]
                    for sub in range(2):
                        tt = k * 2 + sub
                        for hp in range(2):
                            lo, hi = hp * DK, (hp + 1) * DK
                            nc.tensor.matmul(
                                sc[hp][:, sub * SBLK:(sub + 1) * SBLK],
                                lhsT=kwt[pair][lo:hi, tt * P:(tt + 1) * P],
                                rhs=qwt[pair][lo:hi, b * SBLK:(b + 1) * SBLK],
                                start=True, stop=True)
                    ex = [expp.tile([P, 2, SBLK], FP32R, tag=f"exp{hp}", name=f"exp{hp}", bufs=3)
                          for hp in range(2)]
                    for hp in range(2):
                        nc.scalar.activation(
                            ex[hp][:], sc[hp][:].rearrange("p (u q) -> p u q", u=2),
                            mybir.ActivationFunctionType.Exp)
                    exs[k] = ex
                if fillers:
                    fillers.pop(0)()
                # ctx trails exp by 2 steps: the block's first ctx matmul waits
                # for the ct-psum slot freed by the PREVIOUS block's normalize,
                # so give that chain two steps of slack.
                kc = k - 2
                if kc >= 0:
                    ex = exs.pop(kc)
                    for sub in range(2):
                        tt = kc * 2 + sub
                        for hp in range(2):
                            hh = 2 * pair + hp
                            nc.tensor.matmul(
                                ct[hp][:], lhsT=vw[:, tt, hh * (DV + 1):(hh + 1) * (DV + 1)],
                                rhs=ex[hp][:, sub, :],
                                start=(tt == 0), stop=(tt == NTT - 1))

        def attn_normalize(pair, b, ct):
            # ctx = ct[0:64] * (1 / ct[64]) row-broadcast
            for hp in range(2):
                den = smallp.tile([1, SBLK], FP32R, tag="den")
                nc.vector.tensor_copy(den[:], ct[hp][DV:DV + 1, :])
                rb = psum.tile([DV, SBLK], FP32, tag="pj", name="rb")
                nc.tensor.matmul(rb[:], lhsT=ones_r[:, 0:DV], rhs=den[:],
                                 start=True, stop=True)
                rcp = smallp.tile([DV, SBLK], FP32, tag="rcp")
                nc.vector.reciprocal_approx_fast(rcp[:], rb[:])
                nc.vector.tensor_mul(
                    ctx_t[pair][hp * DV:(hp + 1) * DV, b * SBLK:(b + 1) * SBLK],
                    ct[hp][0:DV, :], rcp[:])

        def out_proj_nh(b, st, nh, tag="po"):
            off = b * SBLK + st * P
            po = psum.tile([P, SBLK], FP32, tag=tag, name="po")
            for jc in range(2):
                nc.tensor.matmul(po[:],
                                 lhsT=ctx_t[jc][:, off:off + P],
                                 rhs=wo_sb[:, jc, nh * SBLK:(nh + 1) * SBLK],
                                 start=(jc == 0), stop=(jc == 1))
            ob = outp.tile([P, SBLK], FP32, tag="ob")
            nc.scalar.copy(ob[:], po[:])
            nc.sync.dma_start(out_d[off:off + P, nh * SBLK:(nh + 1) * SBLK], ob[:])

        def proj_qk_piece(xt, w_sb, dst, bias_col, ci, pair, dc_range, pq_holder):
            if dc_range[0] == 0:
                pq_holder[pair] = psum.tile([P, SBLK], FP32, tag="pj", name="pq")
            pq = pq_holder[pair]
            for dc in dc_range:
                nc.tensor.matmul(pq[:], lhsT=w_sb[:, dc, pair * P:(pair + 1) * P],
                                 rhs=xt[:, dc, :], start=(dc == 0), stop=(dc == NDC - 1))
            if dc_range[-1] == NDC - 1:
                nc.scalar.activation(dst[pair][:, ci * SBLK:(ci + 1) * SBLK], pq[:],
                                     mybir.ActivationFunctionType.Identity,
                                     bias=bqk[:, bias_col + pair:bias_col + pair + 1])

        # ---- emission schedule ----
        # K and V fully first (attention needs full-T KWT/VW); Q chunk-by-chunk.
        # The next chunk's Q projection and the previous block's output
        # projection are drained into attention's per-step PE slack.
        vts = {}
        for ci in range(NBLK):
            kt = load_chunk("kt", ci)
            if ci == 0:
                load_w(wv_sb, "wv")
                vts[0] = load_chunk("vt", 0, tag="xtv", bufs=1)
            if ci == 2:
                load_w(wq_sb, "wq")
            proj_qk_pair(kt, wk_sb, kwt, 2, ci, 0)
            proj_qk_pair(kt, wk_sb, kwt, 2, ci, 1)
        for ci in range(NBLK):
            vt = vts.pop(ci) if ci in vts else load_chunk("vt", ci, tag="xtv", bufs=1)
            if ci == 0:
                load_w(wo_sb, "wo", "(jc p) n -> p jc n")
            proj_v(vt, ci)
        qt = load_chunk("qt", 0)
        proj_qk_pair(qt, wq_sb, qwt, 0, 0, 0)
        proj_qk_pair(qt, wq_sb, qwt, 0, 0, 1)
        def interleave(a, bl):
            out = []
            for i in range(max(len(a), len(bl))):
                if i < len(a):
                    out.append(a[i])
                if i < len(bl):
                    out.append(bl[i])
            return out

        prev_norm = None  # pair-1 normalize deferred into the next block
        for b in range(NBLK):
            have_next = b + 1 < NBLK
            pp = [[], []]
            if have_next:
                qt = load_chunk("qt", b + 1)
                holder = [None, None]
                for pair in range(2):
                    for dcs in ([0, 1], [2, 3], [4, 5], [6, 7]):
                        pp[pair].append(lambda xt=qt, p=pair, r=tuple(dcs), h=holder:
                                        proj_qk_piece(xt, wq_sb, qwt, 0, b + 1, p, r, h))
            op = [[], []]
            if b > 0:
                for st in range(4):
                    for nh in range(2):
                        op[st // 2].append(lambda s=st, n=nh: out_proj_nh(b - 1, s, n))
            fill0 = ([prev_norm] if prev_norm else []) + interleave(pp[0], op[0])
            ct0 = attn_alloc(0)
            attn_block(0, b, ct0, fill0)
            fill1 = [lambda bb=b, c=ct0: attn_normalize(0, bb, c)] + interleave(pp[1], op[1])
            ct1 = attn_alloc(1)
            attn_block(1, b, ct1, fill1)
            prev_norm = (lambda bb=b, c=ct1: attn_normalize(1, bb, c))
        prev_norm()
        for st in range(4):
            for nh in range(2):
                out_proj_nh(NBLK - 1, st, nh, tag="po" if (st * 2 + nh) % 2 == 0 else "pj")

    nc.compile()
    return nc


_NC_CACHE = None


def _get_nc():
    global _NC_CACHE
    if _NC_CACHE is None:
        _NC_CACHE = _build_nc()
    return _NC_CACHE


def kernel(Q, K, V, Wq, bq, Wk, bk, Wv, bv, Wo, bo, _trace=False, _trace_kwargs=None):
    nc = _get_nc()
    ones = np.ones((1, SBLK), dtype=np.float32)
    ident = np.eye(P, dtype=np.float32)
    qt_h = [np.ascontiguousarray(np.asarray(Q[b]).T) for b in range(B)]
    kt_h = [np.ascontiguousarray(np.asarray(K[b]).T) for b in range(B)]
    vt_h = [np.ascontiguousarray(np.asarray(V[b]).T) for b in range(B)]

    in_maps = []
    for c in range(N_CORES):
        b, g = c % B, c // B
        hs = list(range(g * HPC, (g + 1) * HPC))
        wq_p = np.concatenate([Wq[h] for h in hs], axis=1)
        wk_p = np.concatenate([Wk[h] for h in hs], axis=1)
        wv_p = np.concatenate([Wv[h] * SCALE for h in hs], axis=1)
        bqk_p = np.stack([
            np.concatenate([bq[hs[0]], bq[hs[1]]]),
            np.concatenate([bq[hs[2]], bq[hs[3]]]),
            np.concatenate([bk[hs[0]], bk[hs[1]]]),
            np.concatenate([bk[hs[2]], bk[hs[3]]]),
            np.concatenate([bv[hs[0]], bv[hs[1]]]) * SCALE,
            np.concatenate([bv[hs[2]], bv[hs[3]]]) * SCALE,
        ], axis=1)
        in_maps.append({
            "qt": qt_h[b], "kt": kt_h[b], "vt": vt_h[b],
            "wq": np.ascontiguousarray(wq_p),
            "wk": np.ascontiguousarray(wk_p),
            "wv": np.ascontiguousarray(wv_p),
            "bqk": np.ascontiguousarray(bqk_p.astype(np.float32)),
            "ident": ident,
            "onescol": np.ones((P, NTT), dtype=np.float32),
            "wo": np.ascontiguousarray(Wo[g * HPC * DV:(g + 1) * HPC * DV]),
            "ones": ones,
        })

    kw = {}
    if _trace:
        kw = dict(trace=True, **(_trace_kwargs or {}))
    res = run_bass_kernel_spmd(nc, in_maps, core_ids=list(range(N_CORES)), **kw)

    out = np.zeros((B, S, D), dtype=np.float32)
    for c in range(N_CORES):
        out[c % B] += res.results[c]["out"]
    out += bo[None, None, :]
    if _trace:
        return out, res
    return out



# revision 8
# speedup vs baseline: 1.4581x; 1.4581x over previous
"""Multi-head attention (B=2, S=2048, D=1024, H=16, dk=dv=64) on 8 TRN2 cores.

Sharding: core c -> batch b = c % 2, head-group g = c // 2 (heads 4g..4g+3).
Each core computes its 4 heads' attention for one batch plus the partial
output projection; the host sums the 4 partials per batch and adds bo plus
the (constant) V-bias term SCALE*bv@Wo -- softmax weights sum to 1, so the
V-bias contributes a constant vector that never needs to touch the device.

Device pipeline (matmuls in float32r: full rate at free>=256, ~13-bit
mantissa). Structured to keep the PE datapath gapless (HAM stays at 2.4GHz)
and the scalar engine saturated with exp:

  1. K proj: kt chunks [128,8,512] -> kwt[pair][dk,S] (heads pair-stacked on
     partitions, bias fused into the ACT PSUM->SBUF eviction). Q proj block 0.
  2. V proj in NATURAL orientation: lhsT = vt chunk (D on partitions),
     rhs = wv -> vw[t, 4*(dv+1)] directly, no PE transposes. The 65th column
     per head is a preloaded ones column (softmax denominator trick).
     Emitted as PE fillers inside the first attention block.
  3. Attention per (pair, block): 16 t-tile steps. Each step:
     scores[t,s] via 2 row-tiled concurrent 64-contraction matmuls into a
     double-buffered 2-bank PSUM tile; ONE exp ACT [128,1024] evicts both
     heads; ctx accumulation trails by 2 steps (2 matmuls into per-pair ct
     banks, ones column gives the denominator row). Fillers (V proj, next
     block's Q proj, previous block's out proj, normalize) drain one per step.
  4. Normalize: denominator row -> K=1 matmul broadcast -> DVE reciprocal ->
     DVE multiply into persistent ctxT. Out proj [s,D] partial with DVE
     eviction, DMA'd out.
"""
import os
import sys

sys.path.insert(0, "/opt/trn_rl_repo")
os.environ.setdefault("JAX_PLATFORMS", "axon,cpu")

from contextlib import ExitStack

import numpy as np

import concourse.bacc as bacc
import concourse.tile as tile
from concourse import mybir
from concourse.bass_utils import run_bass_kernel_spmd

FP32 = mybir.dt.float32
FP32R = mybir.dt.float32r

B, S, D = 2, 2048, 1024
H, DK, DV = 16, 64, 64
N_CORES = 8
HPC = H // (N_CORES // B)  # heads per core = 4
P = 128
SBLK = 512                # s-block (free dim of scores matmuls)
NBLK = S // SBLK          # 4
NTT = S // P              # 16 t-tiles
NDC = D // P              # 8 contraction chunks
NV = HPC * (DV + 1)       # 260
SCALE = 1.0 / (DK * 2.0)  # folded into Wv on host
LAG = 3                   # ctx trails exp by LAG t-tile steps


def _build_nc():
    nc = bacc.Bacc("TRN2", target_bir_lowering=False, debug=False,
                   num_devices=N_CORES)
    d = {}
    for name, shape in [
        ("qt", [D, S]), ("kt", [D, S]), ("vt", [D, S]),
        ("wq", [D, 2 * P]), ("wk", [D, 2 * P]), ("wv", [D, 2 * P]),
        ("bqk", [P, 4]), ("onescol", [P, NTT]),
        ("wo", [HPC * DV, D]), ("ones", [1, SBLK]),
    ]:
        d[name] = nc.dram_tensor(name, shape, FP32, kind="ExternalInput").ap()
    out_d = nc.dram_tensor("out", [S, D], FP32, kind="ExternalOutput").ap()
    xt_view = {
        n: d[n].rearrange("(dc p) s -> p dc s", p=P).bitcast(FP32R)
        for n in ("qt", "kt", "vt")
    }

    with tile.TileContext(nc) as tc, ExitStack() as ctx:
        const = ctx.enter_context(tc.tile_pool(name="const", bufs=1))
        wpool = ctx.enter_context(tc.tile_pool(name="wpool", bufs=1))
        xtp = ctx.enter_context(tc.tile_pool(name="xtp", bufs=4))
        projp = ctx.enter_context(tc.tile_pool(name="projp", bufs=1))
        expp = ctx.enter_context(tc.tile_pool(name="expp", bufs=1))
        ctxp = ctx.enter_context(tc.tile_pool(name="ctxp", bufs=1))
        outp = ctx.enter_context(tc.tile_pool(name="outp", bufs=2))
        smallp = ctx.enter_context(tc.tile_pool(name="smallp", bufs=2))
        psum = ctx.enter_context(tc.tile_pool(name="psum", bufs=1, space="PSUM"))

        # ---- constants / weights (wk first: K projection starts the kernel) ----
        wk_sb = wpool.tile([P, NDC, 2 * P], FP32R)
        nc.sync.dma_start(wk_sb[:], d["wk"].rearrange("(dc p) m -> p dc m", p=P).bitcast(FP32R))
        bqk = const.tile([P, 4], FP32)
        nc.sync.dma_start(bqk[:], d["bqk"])
        ones_r = const.tile([1, SBLK], FP32R)
        nc.sync.dma_start(ones_r[:], d["ones"].bitcast(FP32R))
        wq_sb = wpool.tile([P, NDC, 2 * P], FP32R)
        wv_sb = wpool.tile([P, NDC, 2 * P], FP32R)
        wo_sb = wpool.tile([P, 2, D], FP32R)

        def load_w(sb, name, pat="(dc p) m -> p dc m"):
            nc.sync.dma_start(sb[:], d[name].rearrange(pat, p=P).bitcast(FP32R))

        # ---- persistent activation tiles ----
        qwt = [projp.tile([P, S], FP32R, tag=f"qwt{p_}", name=f"qwt{p_}") for p_ in range(2)]
        kwt = [projp.tile([P, S], FP32R, tag=f"kwt{p_}", name=f"kwt{p_}") for p_ in range(2)]
        vw = projp.tile([P, NTT, NV], FP32R, tag="vw")
        # softmax-denominator ones column (once, strided over the 65-wide head slots)
        for hh in range(HPC):
            nc.sync.dma_start(vw[:, :, hh * (DV + 1) + DV],
                              d["onescol"].bitcast(FP32R))
        ctx_t = [ctxp.tile([P, S], FP32R, tag=f"ctx{p_}", name=f"ctx{p_}") for p_ in range(2)]

        def load_chunk(name, ci):
            xt = xtp.tile([P, NDC, SBLK], FP32R, tag="xt", name="xt")
            nc.sync.dma_start(xt[:], xt_view[name][:, :, ci * SBLK:(ci + 1) * SBLK])
            return xt

        def proj_qk_pair(xt, w_sb, dst, bias_col, ci, pair, tag):
            """Project one head-pair of a 512-chunk into dst[pair][:, ci*SBLK:...]."""
            pq = psum.tile([P, SBLK], FP32, tag=tag, name="pq")
            for dc in range(NDC):
                nc.tensor.matmul(pq[:], lhsT=w_sb[:, dc, pair * P:(pair + 1) * P],
                                 rhs=xt[:, dc, :], start=(dc == 0), stop=(dc == NDC - 1))
            nc.scalar.activation(dst[pair][:, ci * SBLK:(ci + 1) * SBLK], pq[:],
                                 mybir.ActivationFunctionType.Identity,
                                 bias=bqk[:, bias_col + pair:bias_col + pair + 1])

        def proj_qk_piece(xt, w_sb, dst, bias_col, ci, pair, dc_range, pq_holder):
            if dc_range[0] == 0:
                pq_holder[pair] = psum.tile([P, SBLK], FP32, tag="pj", name="pq")
            pq = pq_holder[pair]
            for dc in dc_range:
                nc.tensor.matmul(pq[:], lhsT=w_sb[:, dc, pair * P:(pair + 1) * P],
                                 rhs=xt[:, dc, :], start=(dc == 0), stop=(dc == NDC - 1))
            if dc_range[-1] == NDC - 1:
                nc.scalar.activation(dst[pair][:, ci * SBLK:(ci + 1) * SBLK], pq[:],
                                     mybir.ActivationFunctionType.Identity,
                                     bias=bqk[:, bias_col + pair:bias_col + pair + 1])

        # V proj, natural orientation: one t-tile per call (8 matmuls, free=256).
        # vp holds 2 t-tiles per PSUM bank; evicted per t-tile by the DVE.
        vp_holder = [None]

        def proj_v_tt(vt_chunk, tt):
            par = tt % 2
            if par == 0:
                vp_holder[0] = psum.tile([P, 2, 2 * P], FP32, tag="po", name="vp")
            vp = vp_holder[0]
            off = (tt * P) % SBLK
            for dc in range(NDC):
                nc.tensor.matmul(vp[:, par, :],
                                 lhsT=vt_chunk[:, dc, off:off + P],
                                 rhs=wv_sb[:, dc, :],
                                 start=(dc == 0), stop=(dc == NDC - 1))
            # evict into the 65-wide head slots (dv 0:64 of each slot)
            nc.vector.tensor_copy(
                vw[:, tt, :].rearrange("p (h v) -> p h v", v=DV + 1)[:, :, 0:DV],
                vp[:, par, :].rearrange("p (h v) -> p h v", h=HPC))

        def attn_normalize_hp(pair, b, ct, hp):
            # ctx = ct[0:64] * (1 / ct[64]) row-broadcast, one head
            den = smallp.tile([1, SBLK], FP32R, tag="den")
            nc.vector.tensor_copy(den[:], ct[hp][DV:DV + 1, :])
            rb = psum.tile([DV, SBLK], FP32, tag="pj", name="rb")
            nc.tensor.matmul(rb[:], lhsT=ones_r[:, 0:DV], rhs=den[:],
                             start=True, stop=True)
            rcp = smallp.tile([DV, SBLK], FP32, tag="rcp")
            nc.vector.reciprocal_approx_fast(rcp[:], rb[:])
            nc.vector.tensor_mul(
                ctx_t[pair][hp * DV:(hp + 1) * DV, b * SBLK:(b + 1) * SBLK],
                ct[hp][0:DV, :], rcp[:])

        def norm_fillers(pair, b, ct):
            return [lambda h=hp: attn_normalize_hp(pair, b, ct, h)
                    for hp in range(2)]

        def out_proj_nh(b, st, nh, tag="po"):
            off = b * SBLK + st * P
            po = psum.tile([P, SBLK], FP32, tag=tag, name="po")
            for jc in range(2):
                nc.tensor.matmul(po[:],
                                 lhsT=ctx_t[jc][:, off:off + P],
                                 rhs=wo_sb[:, jc, nh * SBLK:(nh + 1) * SBLK],
                                 start=(jc == 0), stop=(jc == 1))
            ob = outp.tile([P, SBLK], FP32, tag="ob")
            nc.vector.tensor_copy(ob[:], po[:])
            nc.sync.dma_start(out_d[off:off + P, nh * SBLK:(nh + 1) * SBLK], ob[:])

        # ---- prologue: K full, Q block 0 ----
        for ci in range(NBLK):
            kt = load_chunk("kt", ci)
            if ci == 0:
                load_w(wq_sb, "wq")
            if ci == 1:
                load_w(wv_sb, "wv")
            if ci == 2:
                load_w(wo_sb, "wo", "(jc p) n -> p jc n")
            proj_qk_pair(kt, wk_sb, kwt, 2, ci, 0, "pj")
            proj_qk_pair(kt, wk_sb, kwt, 2, ci, 1, "po")
        qt = load_chunk("qt", 0)
        proj_qk_pair(qt, wq_sb, qwt, 0, 0, 0, "pj")
        proj_qk_pair(qt, wq_sb, qwt, 0, 0, 1, "po")
        # vt chunks stream in during the first attention block
        vts = [load_chunk("vt", ci) for ci in range(2)]

        def interleave(a, bl):
            out = []
            for i in range(max(len(a), len(bl))):
                if i < len(a):
                    out.append(a[i])
                if i < len(bl):
                    out.append(bl[i])
            return out

        # ---- attention: (pair, block) segments of 16 t-tile steps ----
        def attn_segment(pair, b, ct, fillers):
            """scores(tt) -> exp(tt) -> ctx(tt-LAG), one filler per step."""
            exs = {}
            for k in range(NTT + LAG):
                if k < NTT:
                    tt = k
                    sc = psum.tile([P, 2, SBLK], FP32, tag="sc", name="sc", bufs=2)
                    for hp in range(2):
                        lo, hi = hp * DK, (hp + 1) * DK
                        nc.tensor.matmul(
                            sc[:, hp, :],
                            lhsT=kwt[pair][lo:hi, tt * P:(tt + 1) * P],
                            rhs=qwt[pair][lo:hi, b * SBLK:(b + 1) * SBLK],
                            start=True, stop=True)
                    ex = expp.tile([P, 2, SBLK], FP32R, tag="exp", name="ex", bufs=LAG + 3)
                    nc.scalar.activation(ex[:], sc[:],
                                         mybir.ActivationFunctionType.Exp)
                    exs[k] = ex
                if fillers:
                    fillers.pop(0)()
                kc = k - LAG
                if kc >= 0:
                    ex = exs.pop(kc)
                    for hp in range(2):
                        hh = 2 * pair + hp
                        nc.tensor.matmul(
                            ct[hp][:], lhsT=vw[:, kc, hh * (DV + 1):(hh + 1) * (DV + 1)],
                            rhs=ex[:, hp, :],
                            start=(kc == 0), stop=(kc == NTT - 1))

        def attn_alloc():
            return [psum.tile([DV + 1, SBLK], FP32, tag=f"ct{hp}", name=f"ct{hp}")
                    for hp in range(2)]

        def v_filler(tt):
            def go():
                ci = tt // (SBLK // P)
                if tt % (SBLK // P) == 0 and ci + 2 < NBLK:
                    vts.append(load_chunk("vt", ci + 2))
                proj_v_tt(vts[ci], tt)
            return go

        prev_norm = []  # pair-1 normalize deferred into the next block
        for b in range(NBLK):
            have_next = b + 1 < NBLK
            # fillers for pair 0 segment: deferred normalize first (ct banks
            # are reused by this segment's ctx at step LAG)
            fill0 = list(prev_norm)
            prev_norm = []
            if b == 0:
                # V projection: all 16 t-tiles as fillers here — filler tt
                # runs at step tt, ctx(tt) consumes vw[tt] at step tt+LAG
                fill0 += [v_filler(tt) for tt in range(NTT)]
            else:
                for st in range(2):
                    for nh in range(2):
                        fill0.append(lambda s=st, n=nh, bb=b: out_proj_nh(bb - 1, s, n))
            ct = attn_alloc()
            attn_segment(0, b, ct, fill0)

            # fillers for pair 1 segment: normalize(0,b) first, then V/q-proj/
            # out-proj pieces
            fill1 = norm_fillers(0, b, ct)
            pp = []
            if have_next:
                qt = load_chunk("qt", b + 1)
                holder = [None, None]
                for pair_ in range(2):
                    for dcs in ([0, 1], [2, 3], [4, 5], [6, 7]):
                        pp.append(lambda xt=qt, p=pair_, r=tuple(dcs), h=holder:
                                  proj_qk_piece(xt, wq_sb, qwt, 0, b + 1, p, r, h))
            op = []
            if b > 0:
                for st in range(2, 4):
                    for nh in range(2):
                        op.append(lambda s=st, n=nh, bb=b: out_proj_nh(bb - 1, s, n))
            ct1 = attn_alloc()
            attn_segment(1, b, ct1, fill1 + interleave(pp, op))
            prev_norm = norm_fillers(1, b, ct1)
        for f in prev_norm:
            f()
        for st in range(4):
            for nh in range(2):
                out_proj_nh(NBLK - 1, st, nh, tag="po" if (st * 2 + nh) % 2 == 0 else "pj")

    nc.compile()
    return nc


_NC_CACHE = None


def _get_nc():
    global _NC_CACHE
    if _NC_CACHE is None:
        _NC_CACHE = _build_nc()
    return _NC_CACHE


def kernel(Q, K, V, Wq, bq, Wk, bk, Wv, bv, Wo, bo, _trace=False, _trace_kwargs=None):
    nc = _get_nc()
    ones = np.ones((1, SBLK), dtype=np.float32)
    qt_h = [np.ascontiguousarray(np.asarray(Q[b]).T) for b in range(B)]
    kt_h = [np.ascontiguousarray(np.asarray(K[b]).T) for b in range(B)]
    vt_h = [np.ascontiguousarray(np.asarray(V[b]).T) for b in range(B)]

    in_maps = []
    for c in range(N_CORES):
        b, g = c % B, c // B
        hs = list(range(g * HPC, (g + 1) * HPC))
        wq_p = np.concatenate([Wq[h] for h in hs], axis=1)
        wk_p = np.concatenate([Wk[h] for h in hs], axis=1)
        wv_p = np.concatenate([Wv[h] * SCALE for h in hs], axis=1)
        bqk_p = np.stack([
            np.concatenate([bq[hs[0]], bq[hs[1]]]),
            np.concatenate([bq[hs[2]], bq[hs[3]]]),
            np.concatenate([bk[hs[0]], bk[hs[1]]]),
            np.concatenate([bk[hs[2]], bk[hs[3]]]),
        ], axis=1)
        in_maps.append({
            "qt": qt_h[b], "kt": kt_h[b], "vt": vt_h[b],
            "wq": np.ascontiguousarray(wq_p),
            "wk": np.ascontiguousarray(wk_p),
            "wv": np.ascontiguousarray(wv_p),
            "bqk": np.ascontiguousarray(bqk_p.astype(np.float32)),
            "onescol": np.ones((P, NTT), dtype=np.float32),
            "wo": np.ascontiguousarray(Wo[g * HPC * DV:(g + 1) * HPC * DV]),
            "ones": ones,
        })

    kw = {}
    if _trace:
        kw = dict(trace=True, **(_trace_kwargs or {}))
    res = run_bass_kernel_spmd(nc, in_maps, core_ids=list(range(N_CORES)), **kw)

    out = np.zeros((B, S, D), dtype=np.float32)
    for c in range(N_CORES):
        out[c % B] += res.results[c]["out"]
    # host-side constant terms: output bias + V-bias (softmax weights sum to 1,
    # so the V bias contributes SCALE * bv @ Wo, constant over (b, s))
    out += bo[None, None, :] + (SCALE * bv.reshape(-1)) @ Wo
    if _trace:
        return out, res
    return out
